# revision 28
# baseline (speedup 1.0000x reference)
"""Mixture-of-Experts (top-2 of 8 experts, erf-GELU FFN) on 8 Trainium2
NeuronCores.

Sharding: experts are grouped (NGRP=2 groups of NSLOT=4 experts) and the
ffn dim F is split NSPLIT=4 ways within a group, so core = (group,
F-slice).  Each core processes the padded token union of its group's
experts against its F-slice of those experts' W1/W2, producing a partial y
(summed over F-slices on the host).  Grouping 4 experts per group lets the
per-group capacity average out routing imbalance: capacity =
sum_i max_g(count of rank-i expert) ~= 2081 vs ideal 2048 (1.6% padding,
vs 5% for pair-groups).

Host side (inside kernel()): router softmax + top-2 + renormalized combine
weights, token dispatch (gather per expert slot), combine (scatter-add of
the F-slice partial sums scaled by the combine weight — the device never
sees wg).  b2 is applied on the host.

Device side (Bass/Tile SPMD), NB_F = 8 F-blocks per core, processed as two
fb-quad passes so MM1 only ever holds 4 PSUM banks per chunk and the
pass-1 DMA need (x once + half of w1) fits the sync queue:
  MM1 (per quad, chunk-major, fb inner):
      h[fb] = gelu(sum_db w1[s,fb,db]^T @ x[chunk] + b1[s,fb])
  MM2 (dt-major):  y[dt] partial = sum_fb w2[s,fb,dt]^T @ h[fb]
bf16 matmuls, fp32 PSUM accumulation, bf16 partial-y output.

Layouts shipped per core (P=128 partitions, C = padded group capacity):
  x   [P, NB_D*C] bf16  chunk-blocked x^T: chunk k holds
                        [p, db, c] = x[off_k+c, db*128+p]
  w1  [NSLOT, P, NB_F, NB_D, P]  [s,p,fb,db,m] = W1[e_s][db*128+p, Foff+fb*128+m]
  w2  [NB_D, P, NSLOT, NB_F, P]  [dt,k,s,fb,m] = W2[e_s][Foff+fb*128+k, dt*128+m]
  b1  [P, NSLOT*NB_F] f32        [p, s*NB_F+fb] = b1[e_s][Foff+fb*128+p]
  out [P, NB_D, C] bf16          partial y^T (unweighted)

Queues (lesson: never mix DMAs into a queue whose engine runs
latency-critical evictions, and a gated DMA must not sit ahead of ungated
ones in a FIFO): sync carries ALL input DMA hand-ordered in consumption
order — x chunks interleaved with the quad-0 w1 slabs, then the quad-1
slabs, then w2 dt-slabs (ring) during MM2; scalar carries only the MM1
gelu evictions and then the MM2 out-group DMAs; vector does the MM2
PSUM->bf16 casts; gpsimd only b1.  x stays fully resident (no ring gates).

The kernel opens with ~20 dummy matmuls on a memset scratch tile: the PE's
HAM clock gate holds the array at 1.2 GHz until it has seen a full ~3.4 us
busy window, and it RE-throttles if the early real matmuls run as
DMA-gated fragments — so the warmup spans the DMA ramp (~1.3 MB must land
first) and real matmuls start gapless at full clock.
"""

import itertools

import numpy as np
import ml_dtypes

P = 128
N_CORES = 8
D, F, E = 1024, 4096, 8
NB_D = D // P
TOP_K = 2

NGRP = 1                 # expert groups (1 = pure F-split, zero padding)
NSLOT = E // NGRP        # experts per group
NSPLIT = N_CORES // NGRP # F-split ways within a group
FS = F // NSPLIT         # F slice per core
NB_F = FS // P           # F blocks per core
QUAD = 4                 # fb blocks per MM1 pass
NPASS = NB_F // QUAD     # passes over the chunk grid

MAX_CHUNK = 512          # PSUM bank = 512 fp32 columns
HEAD_CHUNK = 128         # small first chunk: the FIRST transfer crawls at
                         # ~100-150 GB/s (SDMA spin-up) — keep it tiny
NWARM = 15               # ends as the 2-DMA prefix lands (~13.4 us incl.
                         # the ~2.4 us DMA completion/receipt latency)
TAIL_CHUNK = 128         # final MM2 chunk: the kernel's exposed tail

_cache = {}
_last_in_maps = None


def _chunk_plan(S):
    """MM1 chunk grid: slot-major, per-slot even chunks <= MAX_CHUNK.

    Slot 0 leads with a small head chunk: its x block + first w1 tiles are
    the critical DMA prefix before the first real matmul can issue.
    Returns list of (slot, off, cn, xoff) and C.
    """
    chunks = []
    off = 0
    xoff = 0
    for s, sz in enumerate(S):
        sizes = []
        rem = sz
        if s == 0 and HEAD_CHUNK + P < rem <= HEAD_CHUNK + MAX_CHUNK:
            sizes.append(HEAD_CHUNK)
            rem -= HEAD_CHUNK
        n_ch = max(1, -(-rem // MAX_CHUNK))
        c0 = 0
        for i in range(n_ch):
            cn = (rem - c0 + (n_ch - 1 - i)) // (n_ch - i)
            sizes.append(cn)
            c0 += cn
        c0 = 0
        for cn in sizes:
            chunks.append((s, off + c0, cn, xoff))
            xoff += NB_D * cn
            c0 += cn
        off += sz
    return chunks, off


def _chunk_plan2(S, tail_split=False):
    """MM2's chunk grid: per-slot even chunks, no head split.  With
    tail_split (the last dt only), the final chunk is split down to
    TAIL_CHUNK so the final eviction + out DMA (the kernel's exposed tail)
    is small; other dts skip the extra chunk boundary."""
    chunks = []
    off = 0
    for s, sz in enumerate(S):
        n_ch = max(1, -(-sz // MAX_CHUNK))
        c0 = 0
        for i in range(n_ch):
            cn = (sz - c0 + (n_ch - 1 - i)) // (n_ch - i)
            chunks.append((s, off + c0, cn))
            c0 += cn
        off += sz
    if tail_split:
        s, off, cn = chunks[-1]
        if cn > TAIL_CHUNK + 64:
            chunks[-1] = (s, off, cn - TAIL_CHUNK)
            chunks.append((s, off + cn - TAIL_CHUNK, TAIL_CHUNK))
    return chunks


def _plan(counts):
    """Deal experts into NGRP groups of NSLOT, slot-ordered by count desc.

    Returns (groups, S): groups[g] = expert ids in slot order, S[i] =
    padded slot-i capacity = max over groups of the rank-i count.
    """
    counts = np.asarray(counts)
    order = np.argsort(-counts, kind="stable")
    groups = [
        [int(order[i * NGRP + g]) for i in range(NSLOT)] for g in range(NGRP)
    ]
    S = [
        max(int(counts[groups[g][i]]) for g in range(NGRP)) for i in range(NSLOT)
    ]
    return groups, [max(s, 1) for s in S]


def _build(S):
    """Build + compile the per-core SPMD Bass program for slot sizes S."""
    from concourse import bacc
    import concourse.tile as tile
    import concourse.mybir as mybir

    chunks, C = _chunk_plan(S)
    bf16 = mybir.dt.bfloat16
    f32 = mybir.dt.float32
    GELU = mybir.ActivationFunctionType.Gelu

    nc = bacc.Bacc(None, target_bir_lowering=False)
    x_d = nc.dram_tensor("x", [P, NB_D * C], bf16, kind="ExternalInput")
    w1_d = nc.dram_tensor("w1", [NSLOT, P, NB_F, NB_D, P], bf16, kind="ExternalInput")
    w2_d = nc.dram_tensor("w2", [NB_D, P, NSLOT, NB_F, P], bf16, kind="ExternalInput")
    b1_d = nc.dram_tensor("b1", [P, NSLOT * NB_F], f32, kind="ExternalInput")
    out_d = nc.dram_tensor("out", [P, NB_D, C], bf16, kind="ExternalOutput")

    # out groups: runs of MM2 chunks of ~C/2 columns each; the last dt ends
    # on the small TAIL_CHUNK alone so the exposed tail is tiny.
    def _cuts(ch2, last):
        n2 = len(ch2)
        half = n2
        cum = 0
        for j, (s, off, cn) in enumerate(ch2):
            cum += cn
            if cum >= C // 2:
                half = j + 1
                break
        return [half, n2 - 1, n2] if last else [half, n2]

    grids = [
        (_chunk_plan2(S), False),
        (_chunk_plan2(S, tail_split=True), True),
    ]
    group_w = max(
        ch2[c - 1][1] + ch2[c - 1][2] - ch2[lo][1]
        for ch2, last in grids
        for lo, c in zip([0] + _cuts(ch2, last)[:-1], _cuts(ch2, last))
    )

    with tile.TileContext(nc) as tc:
        with (
            tc.tile_pool(name="const", bufs=1) as const,
            tc.tile_pool(name="xp", bufs=1) as xp,
            tc.tile_pool(name="w1p", bufs=1) as w1p,
            tc.tile_pool(name="w2p", bufs=3 if NGRP == 1 else 4) as w2p,
            tc.tile_pool(name="ps1", bufs=5, space="PSUM") as ps1p,
            tc.tile_pool(name="ps2", bufs=3, space="PSUM") as ps2p,
            tc.tile_pool(name="outp", bufs=4 if NGRP == 1 else 5) as outp,
        ):
            b1_t = const.tile([P, NSLOT * NB_F], f32)
            h_t = const.tile([P, NB_F, C], bf16)
            scr_t = const.tile([P, MAX_CHUNK], bf16)

            # PE warm-up (see module docstring): ends right as the first
            # chunk's x + w1 tiles have landed, so real matmuls run gapless
            # at the full 2.4 GHz from the start.
            nc.vector.memset(scr_t[:], 0.0)
            wps = ps2p.tile([P, MAX_CHUNK], f32, name="ps2", tag="ps2")
            for i in range(NWARM):
                nc.tensor.matmul(
                    wps[:],
                    lhsT=scr_t[:, :P],
                    rhs=scr_t[:],
                    start=(i == 0),
                    stop=(i == NWARM - 1),
                )

            x_t = xp.tile([P, NB_D * C], bf16, name="x_t", tag="x_t")
            w1_t = w1p.tile([P, NSLOT, NB_F, NB_D, P], bf16, name="w1", tag="w1")

            nc.gpsimd.dma_start(b1_t[:], b1_d[:])

            def _x_dma(k, n_sub=1):
                s, off, cn, xoff = chunks[k]
                step = NB_D // n_sub * cn
                for i in range(n_sub):
                    nc.sync.dma_start(
                        x_t[:, xoff + i * step : xoff + (i + 1) * step],
                        x_d[:, xoff + i * step : xoff + (i + 1) * step],
                    )

            # Single sync-queue input stream, hand-ordered in consumption
            # order (two uncoordinated queues split the HBM bandwidth 50/50
            # and break the ordering — measured much slower).  The critical
            # prefix is exactly 2 DMAs: x chunk 0, then slot 0's quad-0
            # slab as ONE 1 MB transfer (per-fb splits pay ~0.65 us of
            # issue cost each and land LATER than one big transfer).
            # Quad-0 slabs for later slots interleave with x chunks; quad-1
            # slabs follow; w2 dt-slabs are emitted inside the MM2 loop.
            _x_dma(0)
            nc.sync.dma_start(w1_t[:, 0, :QUAD], w1_d[0][:, :QUAD])
            pend = list(range(1, NSLOT))
            for k in range(1, len(chunks)):
                _x_dma(k)
                s_next = chunks[min(k + 1, len(chunks) - 1)][0]
                while pend and pend[0] <= s_next:
                    s = pend.pop(0)
                    nc.sync.dma_start(w1_t[:, s, :QUAD], w1_d[s][:, :QUAD])
            for s in pend:
                nc.sync.dma_start(w1_t[:, s, :QUAD], w1_d[s][:, :QUAD])
            if QUAD < NB_F:
                for s in range(NSLOT):
                    nc.sync.dma_start(w1_t[:, s, QUAD:], w1_d[s][:, QUAD:])

            # ---- MM1: two fb-quad passes, chunk-major (fb inner), 4 PSUM
            # banks per chunk; the gelu eviction is the only scalar-queue
            # work in this phase.
            for q in range(NPASS):
                for k, (s, off, cn, xoff) in enumerate(chunks):
                    for fb in range(q * QUAD, (q + 1) * QUAD):
                        ps = ps1p.tile([P, MAX_CHUNK], f32)
                        for db in range(NB_D):
                            nc.tensor.matmul(
                                ps[:, :cn],
                                lhsT=w1_t[:, s, fb, db, :],
                                rhs=x_t[:, xoff + db * cn : xoff + (db + 1) * cn],
                                start=(db == 0),
                                stop=(db == NB_D - 1),
                            )
                        nc.scalar.activation(
                            h_t[:, fb, off : off + cn],
                            ps[:, :cn],
                            GELU,
                            bias=b1_t[:, s * NB_F + fb : s * NB_F + fb + 1],
                        )

            # ---- MM2, dt-major: w2 dt-slabs stream just-in-time from a
            # ring on sync; casts evict on vector; out groups DMA on scalar.
            for dt in range(NB_D):
                w2_t = w2p.tile([P, NSLOT, NB_F, P], bf16, name="w2d", tag="w2d")
                nc.sync.dma_start(w2_t[:], w2_d[dt])
                chunks2, last = grids[1 if dt == NB_D - 1 else 0]
                cuts = _cuts(chunks2, last)
                lo = 0
                for cut in cuts:
                    o_t = outp.tile([P, group_w], bf16, name="o", tag="o")
                    a = chunks2[lo][1]
                    for s, off, cn in chunks2[lo:cut]:
                        ps = ps2p.tile([P, MAX_CHUNK], f32, name="ps2", tag="ps2")
                        for fb in range(NB_F):
                            nc.tensor.matmul(
                                ps[:, :cn],
                                lhsT=w2_t[:, s, fb, :],
                                rhs=h_t[:, fb, off : off + cn],
                                start=(fb == 0),
                                stop=(fb == NB_F - 1),
                            )
                        nc.vector.tensor_copy(
                            o_t[:, off - a : off - a + cn], ps[:, :cn]
                        )
                    b = chunks2[cut - 1][1] + chunks2[cut - 1][2]
                    nc.scalar.dma_start(out_d[:, dt, a:b], o_t[:, : b - a])
                    lo = cut

    nc.compile()
    return nc


def _route(x, W_router):
    """Top-2 routing, replicating jax softmax/top_k/renorm semantics."""
    T = x.shape[0]
    logits = x @ np.asarray(W_router, np.float32)
    m = logits.max(axis=1, keepdims=True)
    ex = np.exp(logits - m, dtype=np.float32)
    probs = ex / ex.sum(axis=1, keepdims=True, dtype=np.float32)
    r = np.arange(T)
    i1 = probs.argmax(axis=1)
    masked = probs.copy()
    masked[r, i1] = -np.inf
    i2 = masked.argmax(axis=1)
    p1 = probs[r, i1]
    p2 = probs[r, i2]
    s = p1 + p2
    return i1, i2, p1 / s, p2 / s


def kernel(hidden_states, W_router, W1, b1, W2, b2):
    from concourse.bass_utils import run_bass_kernel_spmd

    B, S_, D_ = hidden_states.shape
    T = B * S_
    x = np.ascontiguousarray(np.asarray(hidden_states, np.float32).reshape(T, D_))

    i1, i2, w1c, w2c = _route(x, W_router)

    idxs, wgts = [], []
    for e in range(E):
        sel1 = i1 == e
        sel2 = i2 == e
        idx = np.nonzero(sel1 | sel2)[0]
        w = np.where(sel1[idx], w1c[idx], w2c[idx]).astype(np.float32)
        idxs.append(idx)
        wgts.append(w)

    counts = [len(ix) for ix in idxs]
    groups, S = _plan(counts)
    chunks, C = _chunk_plan(S)
    offs = np.concatenate([[0], np.cumsum(S)])[:NSLOT]

    key = tuple(S)
    if key not in _cache:
        _cache[key] = _build(S)
    nc = _cache[key]

    bf16 = ml_dtypes.bfloat16
    xb = x.astype(bf16)
    W1f = np.asarray(W1, np.float32)
    W2f = np.asarray(W2, np.float32)
    b1f = np.asarray(b1, np.float32)

    in_maps = [None] * N_CORES
    for g, grp in enumerate(groups):
        # group-shared: chunk-blocked dispatched x^T (padding slots zero)
        xg = np.zeros((C, D), bf16)
        for s, e in enumerate(grp):
            xg[offs[s] : offs[s] + counts[e]] = xb[idxs[e]]
        x_arr = np.empty((P, NB_D * C), bf16)
        for s, off, cn, xoff in chunks:
            x_arr[:, xoff : xoff + NB_D * cn] = (
                xg[off : off + cn]
                .T.reshape(NB_D, P, cn)
                .transpose(1, 0, 2)
                .reshape(P, NB_D * cn)
            )

        for j in range(NSPLIT):
            foff = j * FS
            w1e = np.ascontiguousarray(
                W1f[grp][:, :, foff : foff + FS]
                .astype(bf16)
                .reshape(NSLOT, NB_D, P, NB_F, P)
                .transpose(0, 2, 3, 1, 4)
            )
            w2e = np.ascontiguousarray(
                W2f[grp][:, foff : foff + FS, :]
                .astype(bf16)
                .reshape(NSLOT, NB_F, P, NB_D, P)
                .transpose(3, 2, 0, 1, 4)
            )
            b1e = np.ascontiguousarray(
                b1f[grp][:, foff : foff + FS]
                .reshape(NSLOT, NB_F, P)
                .transpose(2, 0, 1)
                .reshape(P, NSLOT * NB_F)
            )
            in_maps[g * NSPLIT + j] = {
                "x": x_arr,
                "w1": w1e,
                "w2": w2e,
                "b1": b1e,
            }

    global _last_in_maps
    _last_in_maps = in_maps

    res = run_bass_kernel_spmd(nc, in_maps, core_ids=list(range(N_CORES)))

    out = np.zeros((T, D), np.float32)
    b2f = np.asarray(b2, np.float32)
    for g, grp in enumerate(groups):
        acc = np.zeros((P, NB_D, C), np.float32)
        for j in range(NSPLIT):
            acc += np.asarray(res.results[g * NSPLIT + j]["out"], np.float32)
        for s, e in enumerate(grp):
            n = counts[e]
            y = acc[:, :, offs[s] : offs[s] + n].transpose(2, 1, 0).reshape(n, D)
            out[idxs[e]] += wgts[e][:, None] * y
            if b2f[e].any():
                out[idxs[e]] += wgts[e][:, None] * b2f[e][None, :]
    return out.reshape(B, S_, D_).astype(np.float32)


# revision 30
# speedup vs baseline: 1.0109x; 1.0109x over previous
"""Mixture-of-Experts (top-2 of 8 experts, erf-GELU FFN) on 8 Trainium2
NeuronCores.

Sharding: experts are grouped (NGRP=2 groups of NSLOT=4 experts) and the
ffn dim F is split NSPLIT=4 ways within a group, so core = (group,
F-slice).  Each core processes the padded token union of its group's
experts against its F-slice of those experts' W1/W2, producing a partial y
(summed over F-slices on the host).  Grouping 4 experts per group lets the
per-group capacity average out routing imbalance: capacity =
sum_i max_g(count of rank-i expert) ~= 2081 vs ideal 2048 (1.6% padding,
vs 5% for pair-groups).

Host side (inside kernel()): router softmax + top-2 + renormalized combine
weights, token dispatch (gather per expert slot), combine (scatter-add of
the F-slice partial sums scaled by the combine weight — the device never
sees wg).  b2 is applied on the host.

Device side (Bass/Tile SPMD), NB_F = 8 F-blocks per core, processed as two
fb-quad passes so MM1 only ever holds 4 PSUM banks per chunk and the
pass-1 DMA need (x once + half of w1) fits the sync queue:
  MM1 (per quad, chunk-major, fb inner):
      h[fb] = gelu(sum_db w1[s,fb,db]^T @ x[chunk] + b1[s,fb])
  MM2 (dt-major):  y[dt] partial = sum_fb w2[s,fb,dt]^T @ h[fb]
bf16 matmuls, fp32 PSUM accumulation, bf16 partial-y output.

Layouts shipped per core (P=128 partitions, C = padded group capacity):
  x   [P, NB_D*C] bf16  chunk-blocked x^T: chunk k holds
                        [p, db, c] = x[off_k+c, db*128+p]
  w1  [NSLOT, P, NB_F, NB_D, P]  [s,p,fb,db,m] = W1[e_s][db*128+p, Foff+fb*128+m]
  w2  [NB_D, P, NSLOT, NB_F, P]  [dt,k,s,fb,m] = W2[e_s][Foff+fb*128+k, dt*128+m]
  b1  [P, NSLOT*NB_F] f32        [p, s*NB_F+fb] = b1[e_s][Foff+fb*128+p]
  out [P, NB_D, C] bf16          partial y^T (unweighted)

Queues (lesson: never mix DMAs into a queue whose engine runs
latency-critical evictions, and a gated DMA must not sit ahead of ungated
ones in a FIFO): sync carries ALL input DMA hand-ordered in consumption
order — x chunks interleaved with the quad-0 w1 slabs, then the quad-1
slabs, then w2 dt-slabs (ring) during MM2; scalar carries only the MM1
gelu evictions and then the MM2 out-group DMAs; vector does the MM2
PSUM->bf16 casts; gpsimd only b1.  x stays fully resident (no ring gates).

The kernel opens with ~20 dummy matmuls on a memset scratch tile: the PE's
HAM clock gate holds the array at 1.2 GHz until it has seen a full ~3.4 us
busy window, and it RE-throttles if the early real matmuls run as
DMA-gated fragments — so the warmup spans the DMA ramp (~1.3 MB must land
first) and real matmuls start gapless at full clock.
"""

import numpy as np
import ml_dtypes

P = 128
N_CORES = 8
D, F, E = 1024, 4096, 8
NB_D = D // P
TOP_K = 2

NGRP = 1                 # expert groups (1 = pure F-split, zero padding)
NSLOT = E // NGRP        # experts per group
NSPLIT = N_CORES // NGRP # F-split ways within a group
FS = F // NSPLIT         # F slice per core
NB_F = FS // P           # F blocks per core
QUAD = 4                 # fb blocks per MM1 pass
NPASS = NB_F // QUAD     # passes over the chunk grid

MAX_CHUNK = 512          # PSUM bank = 512 fp32 columns
HEAD_CHUNK = 128         # small first chunk: the FIRST transfer crawls at
                         # ~100-150 GB/s (SDMA spin-up) — keep it tiny
NWARM = 13               # ends as the 2-DMA prefix lands (~13.4 us incl.
                         # the ~2.4 us DMA completion/receipt latency)
TAIL_CHUNK = 128         # final MM2 chunk: the kernel's exposed tail

_cache = {}
_last_in_maps = None


def _chunk_plan(S):
    """MM1 chunk grid: slot-major, per-slot even chunks <= MAX_CHUNK.

    Slot 0 leads with a small head chunk: its x block + first w1 tiles are
    the critical DMA prefix before the first real matmul can issue.
    Returns list of (slot, off, cn, xoff) and C.
    """
    chunks = []
    off = 0
    xoff = 0
    for s, sz in enumerate(S):
        sizes = []
        rem = sz
        if s == 0 and HEAD_CHUNK + P < rem <= HEAD_CHUNK + MAX_CHUNK:
            sizes.append(HEAD_CHUNK)
            rem -= HEAD_CHUNK
        n_ch = max(1, -(-rem // MAX_CHUNK))
        c0 = 0
        for i in range(n_ch):
            cn = (rem - c0 + (n_ch - 1 - i)) // (n_ch - i)
            sizes.append(cn)
            c0 += cn
        c0 = 0
        for cn in sizes:
            chunks.append((s, off + c0, cn, xoff))
            xoff += NB_D * cn
            c0 += cn
        off += sz
    return chunks, off


def _chunk_plan2(S, tail_split=False):
    """MM2's chunk grid: per-slot even chunks, no head split.  With
    tail_split (the last dt only), the final chunk is split down to
    TAIL_CHUNK so the final eviction + out DMA (the kernel's exposed tail)
    is small; other dts skip the extra chunk boundary."""
    chunks = []
    off = 0
    for s, sz in enumerate(S):
        n_ch = max(1, -(-sz // MAX_CHUNK))
        c0 = 0
        for i in range(n_ch):
            cn = (sz - c0 + (n_ch - 1 - i)) // (n_ch - i)
            chunks.append((s, off + c0, cn))
            c0 += cn
        off += sz
    if tail_split:
        s, off, cn = chunks[-1]
        if cn > TAIL_CHUNK + 64:
            chunks[-1] = (s, off, cn - TAIL_CHUNK)
            chunks.append((s, off + cn - TAIL_CHUNK, TAIL_CHUNK))
    return chunks


def _plan(counts):
    """Deal experts into NGRP groups of NSLOT, slot-ordered by count desc.

    Returns (groups, S): groups[g] = expert ids in slot order, S[i] =
    padded slot-i capacity = max over groups of the rank-i count.
    """
    counts = np.asarray(counts)
    order = np.argsort(-counts, kind="stable")
    groups = [
        [int(order[i * NGRP + g]) for i in range(NSLOT)] for g in range(NGRP)
    ]
    S = [
        max(int(counts[groups[g][i]]) for g in range(NGRP)) for i in range(NSLOT)
    ]
    return groups, [max(s, 1) for s in S]


def _build(S):
    """Build + compile the per-core SPMD Bass program for slot sizes S."""
    from concourse import bacc
    import concourse.tile as tile
    import concourse.mybir as mybir

    chunks, C = _chunk_plan(S)
    bf16 = mybir.dt.bfloat16
    f32 = mybir.dt.float32
    GELU = mybir.ActivationFunctionType.Gelu

    nc = bacc.Bacc(None, target_bir_lowering=False)
    x_d = nc.dram_tensor("x", [P, NB_D * C], bf16, kind="ExternalInput")
    w1_d = nc.dram_tensor("w1", [NSLOT, P, NB_F, NB_D, P], bf16, kind="ExternalInput")
    w2_d = nc.dram_tensor("w2", [NB_D, P, NSLOT, NB_F, P], bf16, kind="ExternalInput")
    b1_d = nc.dram_tensor("b1", [P, NSLOT * NB_F], f32, kind="ExternalInput")
    out_d = nc.dram_tensor("out", [P, NB_D, C], bf16, kind="ExternalOutput")

    # out groups: runs of MM2 chunks of ~C/2 columns each; the last dt ends
    # on the small TAIL_CHUNK alone so the exposed tail is tiny.
    def _cuts(ch2, last):
        n2 = len(ch2)
        half = n2
        cum = 0
        for j, (s, off, cn) in enumerate(ch2):
            cum += cn
            if cum >= C // 2:
                half = j + 1
                break
        return [half, n2 - 1, n2] if last else [half, n2]

    grids = [
        (_chunk_plan2(S), False),
        (_chunk_plan2(S, tail_split=True), True),
    ]
    group_w = max(
        ch2[c - 1][1] + ch2[c - 1][2] - ch2[lo][1]
        for ch2, last in grids
        for lo, c in zip([0] + _cuts(ch2, last)[:-1], _cuts(ch2, last))
    )

    with tile.TileContext(nc) as tc:
        with (
            tc.tile_pool(name="const", bufs=1) as const,
            tc.tile_pool(name="xp", bufs=1) as xp,
            tc.tile_pool(name="w1p", bufs=1) as w1p,
            tc.tile_pool(name="w2p", bufs=3 if NGRP == 1 else 4) as w2p,
            tc.tile_pool(name="ps1", bufs=5, space="PSUM") as ps1p,
            tc.tile_pool(name="ps2", bufs=3, space="PSUM") as ps2p,
            tc.tile_pool(name="outp", bufs=4 if NGRP == 1 else 5) as outp,
        ):
            b1_t = const.tile([P, NSLOT * NB_F], f32)
            h_t = const.tile([P, NB_F, C], bf16)
            scr_t = const.tile([P, MAX_CHUNK], bf16)

            # PE warm-up (see module docstring): ends right as the first
            # chunk's x + w1 tiles have landed, so real matmuls run gapless
            # at the full 2.4 GHz from the start.
            nc.vector.memset(scr_t[:], 0.0)
            wps = ps2p.tile([P, MAX_CHUNK], f32, name="ps2", tag="ps2")
            for i in range(NWARM):
                nc.tensor.matmul(
                    wps[:],
                    lhsT=scr_t[:, :P],
                    rhs=scr_t[:],
                    start=(i == 0),
                    stop=(i == NWARM - 1),
                )

            x_t = xp.tile([P, NB_D * C], bf16, name="x_t", tag="x_t")
            w1_t = w1p.tile([P, NSLOT, NB_F, NB_D, P], bf16, name="w1", tag="w1")

            nc.gpsimd.dma_start(b1_t[:], b1_d[:])

            def _x_dma(k, n_sub=1):
                s, off, cn, xoff = chunks[k]
                step = NB_D // n_sub * cn
                for i in range(n_sub):
                    nc.sync.dma_start(
                        x_t[:, xoff + i * step : xoff + (i + 1) * step],
                        x_d[:, xoff + i * step : xoff + (i + 1) * step],
                    )

            # Single sync-queue input stream, hand-ordered in consumption
            # order (two uncoordinated queues split the HBM bandwidth 50/50
            # and break the ordering — measured much slower).  The critical
            # prefix is exactly 2 DMAs: x chunk 0, then slot 0's quad-0
            # slab as ONE 1 MB transfer (per-fb splits pay ~0.65 us of
            # issue cost each and land LATER than one big transfer).
            # Quad-0 slabs for later slots interleave with x chunks; quad-1
            # slabs follow; w2 dt-slabs are emitted inside the MM2 loop.
            _x_dma(0)
            nc.sync.dma_start(w1_t[:, 0, :QUAD], w1_d[0][:, :QUAD])
            pend = list(range(1, NSLOT))
            for k in range(1, len(chunks)):
                _x_dma(k)
                s_next = chunks[min(k + 1, len(chunks) - 1)][0]
                while pend and pend[0] <= s_next:
                    s = pend.pop(0)
                    nc.sync.dma_start(w1_t[:, s, :QUAD], w1_d[s][:, :QUAD])
            for s in pend:
                nc.sync.dma_start(w1_t[:, s, :QUAD], w1_d[s][:, :QUAD])
            if QUAD < NB_F:
                for s in range(NSLOT):
                    nc.sync.dma_start(w1_t[:, s, QUAD:], w1_d[s][:, QUAD:])

            # ---- MM1: two fb-quad passes, chunk-major (fb inner), 4 PSUM
            # banks per chunk; the gelu eviction is the only scalar-queue
            # work in this phase.
            for q in range(NPASS):
                for k, (s, off, cn, xoff) in enumerate(chunks):
                    for fb in range(q * QUAD, (q + 1) * QUAD):
                        ps = ps1p.tile([P, MAX_CHUNK], f32)
                        for db in range(NB_D):
                            nc.tensor.matmul(
                                ps[:, :cn],
                                lhsT=w1_t[:, s, fb, db, :],
                                rhs=x_t[:, xoff + db * cn : xoff + (db + 1) * cn],
                                start=(db == 0),
                                stop=(db == NB_D - 1),
                            )
                        nc.scalar.activation(
                            h_t[:, fb, off : off + cn],
                            ps[:, :cn],
                            GELU,
                            bias=b1_t[:, s * NB_F + fb : s * NB_F + fb + 1],
                        )

            # ---- MM2, dt-major: w2 dt-slabs stream just-in-time from a
            # ring on sync; casts evict on vector; out groups DMA on scalar.
            for dt in range(NB_D):
                w2_t = w2p.tile([P, NSLOT, NB_F, P], bf16, name="w2d", tag="w2d")
                nc.sync.dma_start(w2_t[:], w2_d[dt])
                chunks2, last = grids[1 if dt == NB_D - 1 else 0]
                cuts = _cuts(chunks2, last)
                lo = 0
                for cut in cuts:
                    o_t = outp.tile([P, group_w], bf16, name="o", tag="o")
                    a = chunks2[lo][1]
                    for s, off, cn in chunks2[lo:cut]:
                        ps = ps2p.tile([P, MAX_CHUNK], f32, name="ps2", tag="ps2")
                        for fb in range(NB_F):
                            nc.tensor.matmul(
                                ps[:, :cn],
                                lhsT=w2_t[:, s, fb, :],
                                rhs=h_t[:, fb, off : off + cn],
                                start=(fb == 0),
                                stop=(fb == NB_F - 1),
                            )
                        nc.vector.tensor_copy(
                            o_t[:, off - a : off - a + cn], ps[:, :cn]
                        )
                    b = chunks2[cut - 1][1] + chunks2[cut - 1][2]
                    nc.scalar.dma_start(out_d[:, dt, a:b], o_t[:, : b - a])
                    lo = cut

    nc.compile()
    return nc


def _route(x, W_router):
    """Top-2 routing, replicating jax softmax/top_k/renorm semantics."""
    T = x.shape[0]
    logits = x @ np.asarray(W_router, np.float32)
    m = logits.max(axis=1, keepdims=True)
    ex = np.exp(logits - m, dtype=np.float32)
    probs = ex / ex.sum(axis=1, keepdims=True, dtype=np.float32)
    r = np.arange(T)
    i1 = probs.argmax(axis=1)
    masked = probs.copy()
    masked[r, i1] = -np.inf
    i2 = masked.argmax(axis=1)
    p1 = probs[r, i1]
    p2 = probs[r, i2]
    s = p1 + p2
    return i1, i2, p1 / s, p2 / s


def kernel(hidden_states, W_router, W1, b1, W2, b2):
    from concourse.bass_utils import run_bass_kernel_spmd

    B, S_, D_ = hidden_states.shape
    T = B * S_
    x = np.ascontiguousarray(np.asarray(hidden_states, np.float32).reshape(T, D_))

    i1, i2, w1c, w2c = _route(x, W_router)

    idxs, wgts = [], []
    for e in range(E):
        sel1 = i1 == e
        sel2 = i2 == e
        idx = np.nonzero(sel1 | sel2)[0]
        w = np.where(sel1[idx], w1c[idx], w2c[idx]).astype(np.float32)
        idxs.append(idx)
        wgts.append(w)

    counts = [len(ix) for ix in idxs]
    groups, S = _plan(counts)
    chunks, C = _chunk_plan(S)
    offs = np.concatenate([[0], np.cumsum(S)])[:NSLOT]

    key = tuple(S)
    if key not in _cache:
        _cache[key] = _build(S)
    nc = _cache[key]

    bf16 = ml_dtypes.bfloat16
    xb = x.astype(bf16)
    W1f = np.asarray(W1, np.float32)
    W2f = np.asarray(W2, np.float32)
    b1f = np.asarray(b1, np.float32)

    in_maps = [None] * N_CORES
    for g, grp in enumerate(groups):
        # group-shared: chunk-blocked dispatched x^T (padding slots zero)
        xg = np.zeros((C, D), bf16)
        for s, e in enumerate(grp):
            xg[offs[s] : offs[s] + counts[e]] = xb[idxs[e]]
        x_arr = np.empty((P, NB_D * C), bf16)
        for s, off, cn, xoff in chunks:
            x_arr[:, xoff : xoff + NB_D * cn] = (
                xg[off : off + cn]
                .T.reshape(NB_D, P, cn)
                .transpose(1, 0, 2)
                .reshape(P, NB_D * cn)
            )

        for j in range(NSPLIT):
            foff = j * FS
            w1e = np.ascontiguousarray(
                W1f[grp][:, :, foff : foff + FS]
                .astype(bf16)
                .reshape(NSLOT, NB_D, P, NB_F, P)
                .transpose(0, 2, 3, 1, 4)
            )
            w2e = np.ascontiguousarray(
                W2f[grp][:, foff : foff + FS, :]
                .astype(bf16)
                .reshape(NSLOT, NB_F, P, NB_D, P)
                .transpose(3, 2, 0, 1, 4)
            )
            b1e = np.ascontiguousarray(
                b1f[grp][:, foff : foff + FS]
                .reshape(NSLOT, NB_F, P)
                .transpose(2, 0, 1)
                .reshape(P, NSLOT * NB_F)
            )
            in_maps[g * NSPLIT + j] = {
                "x": x_arr,
                "w1": w1e,
                "w2": w2e,
                "b1": b1e,
            }

    global _last_in_maps
    _last_in_maps = in_maps

    res = run_bass_kernel_spmd(nc, in_maps, core_ids=list(range(N_CORES)))

    out = np.zeros((T, D), np.float32)
    b2f = np.asarray(b2, np.float32)
    for g, grp in enumerate(groups):
        acc = np.zeros((P, NB_D, C), np.float32)
        for j in range(NSPLIT):
            acc += np.asarray(res.results[g * NSPLIT + j]["out"], np.float32)
        for s, e in enumerate(grp):
            n = counts[e]
            y = acc[:, :, offs[s] : offs[s] + n].transpose(2, 1, 0).reshape(n, D)
            out[idxs[e]] += wgts[e][:, None] * y
            if b2f[e].any():
                out[idxs[e]] += wgts[e][:, None] * b2f[e][None, :]
    return out.reshape(B, S_, D_).astype(np.float32)


# revision 31
# speedup vs baseline: 1.0125x; 1.0016x over previous
"""Mixture-of-Experts (top-2 of 8 experts, erf-GELU FFN) on 8 Trainium2
NeuronCores.

Sharding: pure tensor-parallel over the FFN dim (NGRP=1) — every core
holds ALL 8 experts but only an F/8 = 512 slice of W1/W2.  The dispatched
token set is identical on all cores (the full top-2 dispatch, C = T*top_k
= 4096 token slots grouped by expert), so per-core capacity is exact with
ZERO load-imbalance padding (an expert-parallel split pays max-over-groups
padding).  Each core computes a partial y over its F-slice; the host sums
the 8 partials, applies the combine weights, and scatter-adds into the
output.  b2 is applied on the host.

Host side (inside kernel()): router softmax + top-2 + renormalized combine
weights, token dispatch (gather per expert slot), combine (scatter-add of
the F-slice partial sums scaled by the combine weight — the device never
sees wg).

Device side (Bass/Tile SPMD), NB_F = 4 F-blocks per core, one chunk-major
pass (fb inner) so MM1 holds only 4 PSUM banks per chunk:
  MM1:  h[fb] = gelu(sum_db w1[s,fb,db]^T @ x[chunk] + b1[s,fb])
  MM2 (dt-major):  y[dt] partial = sum_fb w2[s,fb,dt]^T @ h[fb]
bf16 matmuls, fp32 PSUM accumulation, bf16 partial-y output.

Layouts shipped per core (P=128 partitions, C = 4096):
  x   [P, NB_D*C] bf16  chunk-blocked x^T: chunk k holds
                        [p, db, c] = x[off_k+c, db*128+p]
  w1  [NSLOT, P, NB_F, NB_D, P]  [s,p,fb,db,m] = W1[e_s][db*128+p, Foff+fb*128+m]
  w2  [NB_D, P, NSLOT, NB_F, P]  [dt,k,s,fb,m] = W2[e_s][Foff+fb*128+k, dt*128+m]
  b1  [P, NSLOT*NB_F] f32        [p, s*NB_F+fb] = b1[e_s][Foff+fb*128+p]
  out [P, NB_D, C] bf16          partial y^T (unweighted)

Queues (measured lessons: never mix DMAs into a queue whose engine runs
latency-critical evictions — the FIFO couples them and PSUM backpressure
stalls the PE; and two uncoordinated queues split HBM bandwidth 50/50,
breaking any hand-ordering): sync carries ALL input DMA hand-ordered in
consumption order — x chunks interleaved with per-slot w1 slabs, then w2
dt-slabs (ring) during MM2; scalar carries only the MM1 gelu evictions and
then the MM2 out-group DMAs; vector does the MM2 PSUM->bf16 casts; gpsimd
only b1.  x stays fully resident (no ring gates).  The critical prefix is
2 DMAs (small x chunk 0, then slot 0's whole w1 slab as ONE 1 MB transfer
— merged transfers land earlier than per-fb splits, ~0.65 us issue cost
each); the first transfer crawls (~100-150 GB/s SDMA spin-up) so chunk 0
is kept small.

The kernel opens with 13 dummy matmuls on a memset scratch tile: the PE's
HAM clock gate holds the array at 1.2 GHz until it has seen a full ~3.4 us
busy window, and it RE-throttles if the early real matmuls run as
DMA-gated fragments — so the warmup spans the DMA ramp until the prefix
lands (~13.4 us incl. ~2.4 us completion receipt) and real matmuls start
gapless at full clock.
"""

import numpy as np
import ml_dtypes

P = 128
N_CORES = 8
D, F, E = 1024, 4096, 8
NB_D = D // P
TOP_K = 2

NGRP = 1                 # expert groups (1 = pure F-split, zero padding)
NSLOT = E // NGRP        # experts per group
NSPLIT = N_CORES // NGRP # F-split ways within a group
FS = F // NSPLIT         # F slice per core
NB_F = FS // P           # F blocks per core
QUAD = 4                 # fb blocks per MM1 pass
NPASS = NB_F // QUAD     # passes over the chunk grid

MAX_CHUNK = 512          # PSUM bank = 512 fp32 columns
HEAD_CHUNK = 128         # small first chunk: the FIRST transfer crawls at
                         # ~100-150 GB/s (SDMA spin-up) — keep it tiny
NWARM = 13               # ends as the 2-DMA prefix lands (~13.4 us incl.
                         # the ~2.4 us DMA completion/receipt latency)
TAIL_CHUNK = 128         # final MM2 chunk: the kernel's exposed tail

_cache = {}
_last_in_maps = None


def _chunk_plan(S):
    """MM1 chunk grid: slot-major, per-slot even chunks <= MAX_CHUNK.

    Slot 0 leads with a small head chunk: its x block + first w1 tiles are
    the critical DMA prefix before the first real matmul can issue.
    Returns list of (slot, off, cn, xoff) and C.
    """
    chunks = []
    off = 0
    xoff = 0
    for s, sz in enumerate(S):
        sizes = []
        rem = sz
        if s == 0 and HEAD_CHUNK + P < rem <= HEAD_CHUNK + MAX_CHUNK:
            sizes.append(HEAD_CHUNK)
            rem -= HEAD_CHUNK
        n_ch = max(1, -(-rem // MAX_CHUNK))
        c0 = 0
        for i in range(n_ch):
            cn = (rem - c0 + (n_ch - 1 - i)) // (n_ch - i)
            sizes.append(cn)
            c0 += cn
        c0 = 0
        for cn in sizes:
            chunks.append((s, off + c0, cn, xoff))
            xoff += NB_D * cn
            c0 += cn
        off += sz
    return chunks, off


def _chunk_plan2(S, tail_split=False):
    """MM2's chunk grid: per-slot even chunks, no head split.  With
    tail_split (the last dt only), the final chunk is split down to
    TAIL_CHUNK so the final eviction + out DMA (the kernel's exposed tail)
    is small; other dts skip the extra chunk boundary."""
    chunks = []
    off = 0
    for s, sz in enumerate(S):
        n_ch = max(1, -(-sz // MAX_CHUNK))
        c0 = 0
        for i in range(n_ch):
            cn = (sz - c0 + (n_ch - 1 - i)) // (n_ch - i)
            chunks.append((s, off + c0, cn))
            c0 += cn
        off += sz
    if tail_split:
        s, off, cn = chunks[-1]
        if cn > TAIL_CHUNK + 64:
            chunks[-1] = (s, off, cn - TAIL_CHUNK)
            chunks.append((s, off + cn - TAIL_CHUNK, TAIL_CHUNK))
    return chunks


def _plan(counts):
    """Deal experts into NGRP groups of NSLOT, slot-ordered by count desc.

    Returns (groups, S): groups[g] = expert ids in slot order, S[i] =
    padded slot-i capacity = max over groups of the rank-i count.
    """
    counts = np.asarray(counts)
    order = np.argsort(-counts, kind="stable")
    groups = [
        [int(order[i * NGRP + g]) for i in range(NSLOT)] for g in range(NGRP)
    ]
    S = [
        max(int(counts[groups[g][i]]) for g in range(NGRP)) for i in range(NSLOT)
    ]
    return groups, [max(s, 1) for s in S]


def _build(S):
    """Build + compile the per-core SPMD Bass program for slot sizes S."""
    from concourse import bacc
    import concourse.tile as tile
    import concourse.mybir as mybir

    chunks, C = _chunk_plan(S)
    bf16 = mybir.dt.bfloat16
    f32 = mybir.dt.float32
    GELU = mybir.ActivationFunctionType.Gelu

    nc = bacc.Bacc(None, target_bir_lowering=False)
    x_d = nc.dram_tensor("x", [P, NB_D * C], bf16, kind="ExternalInput")
    w1_d = nc.dram_tensor("w1", [NSLOT, P, NB_F, NB_D, P], bf16, kind="ExternalInput")
    w2_d = nc.dram_tensor("w2", [NB_D, P, NSLOT, NB_F, P], bf16, kind="ExternalInput")
    b1_d = nc.dram_tensor("b1", [P, NSLOT * NB_F], f32, kind="ExternalInput")
    out_d = nc.dram_tensor("out", [P, NB_D, C], bf16, kind="ExternalOutput")

    # out groups: runs of MM2 chunks of ~C/2 columns each; the last dt ends
    # on the small TAIL_CHUNK alone so the exposed tail is tiny.
    def _cuts(ch2, last):
        n2 = len(ch2)
        half = n2
        cum = 0
        for j, (s, off, cn) in enumerate(ch2):
            cum += cn
            if cum >= C // 2:
                half = j + 1
                break
        return [half, n2 - 1, n2] if last else [half, n2]

    grids = [
        (_chunk_plan2(S), False),
        (_chunk_plan2(S, tail_split=True), True),
    ]
    group_w = max(
        ch2[c - 1][1] + ch2[c - 1][2] - ch2[lo][1]
        for ch2, last in grids
        for lo, c in zip([0] + _cuts(ch2, last)[:-1], _cuts(ch2, last))
    )

    with tile.TileContext(nc) as tc:
        with (
            tc.tile_pool(name="const", bufs=1) as const,
            tc.tile_pool(name="xp", bufs=1) as xp,
            tc.tile_pool(name="w1p", bufs=1) as w1p,
            tc.tile_pool(name="w2p", bufs=3 if NGRP == 1 else 4) as w2p,
            tc.tile_pool(name="ps1", bufs=5, space="PSUM") as ps1p,
            tc.tile_pool(name="ps2", bufs=3, space="PSUM") as ps2p,
            tc.tile_pool(name="outp", bufs=4 if NGRP == 1 else 5) as outp,
        ):
            b1_t = const.tile([P, NSLOT * NB_F], f32)
            h_t = const.tile([P, NB_F, C], bf16)
            scr_t = const.tile([P, MAX_CHUNK], bf16)

            # PE warm-up (see module docstring): ends right as the first
            # chunk's x + w1 tiles have landed, so real matmuls run gapless
            # at the full 2.4 GHz from the start.
            nc.vector.memset(scr_t[:], 0.0)
            wps = ps2p.tile([P, MAX_CHUNK], f32, name="ps2", tag="ps2")
            for i in range(NWARM):
                nc.tensor.matmul(
                    wps[:],
                    lhsT=scr_t[:, :P],
                    rhs=scr_t[:],
                    start=(i == 0),
                    stop=(i == NWARM - 1),
                )

            x_t = xp.tile([P, NB_D * C], bf16, name="x_t", tag="x_t")
            w1_t = w1p.tile([P, NSLOT, NB_F, NB_D, P], bf16, name="w1", tag="w1")

            nc.gpsimd.dma_start(b1_t[:], b1_d[:])

            def _x_dma(k, n_sub=1):
                s, off, cn, xoff = chunks[k]
                step = NB_D // n_sub * cn
                for i in range(n_sub):
                    nc.sync.dma_start(
                        x_t[:, xoff + i * step : xoff + (i + 1) * step],
                        x_d[:, xoff + i * step : xoff + (i + 1) * step],
                    )

            # Single sync-queue input stream, hand-ordered in consumption
            # order (two uncoordinated queues split the HBM bandwidth 50/50
            # and break the ordering — measured much slower).  The critical
            # prefix is exactly 2 DMAs: x chunk 0, then slot 0's quad-0
            # slab as ONE 1 MB transfer (per-fb splits pay ~0.65 us of
            # issue cost each and land LATER than one big transfer).
            # Quad-0 slabs for later slots interleave with x chunks; quad-1
            # slabs follow; w2 dt-slabs are emitted inside the MM2 loop.
            _x_dma(0)
            nc.sync.dma_start(w1_t[:, 0, :QUAD], w1_d[0][:, :QUAD])
            pend = list(range(1, NSLOT))
            for k in range(1, len(chunks)):
                _x_dma(k)
                s_next = chunks[min(k + 1, len(chunks) - 1)][0]
                while pend and pend[0] <= s_next:
                    s = pend.pop(0)
                    nc.sync.dma_start(w1_t[:, s, :QUAD], w1_d[s][:, :QUAD])
            for s in pend:
                nc.sync.dma_start(w1_t[:, s, :QUAD], w1_d[s][:, :QUAD])
            if QUAD < NB_F:
                for s in range(NSLOT):
                    nc.sync.dma_start(w1_t[:, s, QUAD:], w1_d[s][:, QUAD:])

            # ---- MM1: two fb-quad passes, chunk-major (fb inner), 4 PSUM
            # banks per chunk; the gelu eviction is the only scalar-queue
            # work in this phase.
            for q in range(NPASS):
                for k, (s, off, cn, xoff) in enumerate(chunks):
                    for fb in range(q * QUAD, (q + 1) * QUAD):
                        ps = ps1p.tile([P, MAX_CHUNK], f32)
                        for db in range(NB_D):
                            nc.tensor.matmul(
                                ps[:, :cn],
                                lhsT=w1_t[:, s, fb, db, :],
                                rhs=x_t[:, xoff + db * cn : xoff + (db + 1) * cn],
                                start=(db == 0),
                                stop=(db == NB_D - 1),
                            )
                        nc.scalar.activation(
                            h_t[:, fb, off : off + cn],
                            ps[:, :cn],
                            GELU,
                            bias=b1_t[:, s * NB_F + fb : s * NB_F + fb + 1],
                        )

            # ---- MM2, dt-major: w2 dt-slabs stream just-in-time from a
            # ring on sync; casts evict on vector; out groups DMA on scalar.
            for dt in range(NB_D):
                w2_t = w2p.tile([P, NSLOT, NB_F, P], bf16, name="w2d", tag="w2d")
                nc.sync.dma_start(w2_t[:], w2_d[dt])
                chunks2, last = grids[1 if dt == NB_D - 1 else 0]
                cuts = _cuts(chunks2, last)
                lo = 0
                for cut in cuts:
                    o_t = outp.tile([P, group_w], bf16, name="o", tag="o")
                    a = chunks2[lo][1]
                    for s, off, cn in chunks2[lo:cut]:
                        ps = ps2p.tile([P, MAX_CHUNK], f32, name="ps2", tag="ps2")
                        for fb in range(NB_F):
                            nc.tensor.matmul(
                                ps[:, :cn],
                                lhsT=w2_t[:, s, fb, :],
                                rhs=h_t[:, fb, off : off + cn],
                                start=(fb == 0),
                                stop=(fb == NB_F - 1),
                            )
                        nc.vector.tensor_copy(
                            o_t[:, off - a : off - a + cn], ps[:, :cn]
                        )
                    b = chunks2[cut - 1][1] + chunks2[cut - 1][2]
                    nc.scalar.dma_start(out_d[:, dt, a:b], o_t[:, : b - a])
                    lo = cut

    nc.compile()
    return nc


def _route(x, W_router):
    """Top-2 routing, replicating jax softmax/top_k/renorm semantics."""
    T = x.shape[0]
    logits = x @ np.asarray(W_router, np.float32)
    m = logits.max(axis=1, keepdims=True)
    ex = np.exp(logits - m, dtype=np.float32)
    probs = ex / ex.sum(axis=1, keepdims=True, dtype=np.float32)
    r = np.arange(T)
    i1 = probs.argmax(axis=1)
    masked = probs.copy()
    masked[r, i1] = -np.inf
    i2 = masked.argmax(axis=1)
    p1 = probs[r, i1]
    p2 = probs[r, i2]
    s = p1 + p2
    return i1, i2, p1 / s, p2 / s


def kernel(hidden_states, W_router, W1, b1, W2, b2):
    from concourse.bass_utils import run_bass_kernel_spmd

    B, S_, D_ = hidden_states.shape
    T = B * S_
    x = np.ascontiguousarray(np.asarray(hidden_states, np.float32).reshape(T, D_))

    i1, i2, w1c, w2c = _route(x, W_router)

    idxs, wgts = [], []
    for e in range(E):
        sel1 = i1 == e
        sel2 = i2 == e
        idx = np.nonzero(sel1 | sel2)[0]
        w = np.where(sel1[idx], w1c[idx], w2c[idx]).astype(np.float32)
        idxs.append(idx)
        wgts.append(w)

    counts = [len(ix) for ix in idxs]
    groups, S = _plan(counts)
    chunks, C = _chunk_plan(S)
    offs = np.concatenate([[0], np.cumsum(S)])[:NSLOT]

    key = tuple(S)
    if key not in _cache:
        _cache[key] = _build(S)
    nc = _cache[key]

    bf16 = ml_dtypes.bfloat16
    xb = x.astype(bf16)
    W1f = np.asarray(W1, np.float32)
    W2f = np.asarray(W2, np.float32)
    b1f = np.asarray(b1, np.float32)

    in_maps = [None] * N_CORES
    for g, grp in enumerate(groups):
        # group-shared: chunk-blocked dispatched x^T (padding slots zero)
        xg = np.zeros((C, D), bf16)
        for s, e in enumerate(grp):
            xg[offs[s] : offs[s] + counts[e]] = xb[idxs[e]]
        x_arr = np.empty((P, NB_D * C), bf16)
        for s, off, cn, xoff in chunks:
            x_arr[:, xoff : xoff + NB_D * cn] = (
                xg[off : off + cn]
                .T.reshape(NB_D, P, cn)
                .transpose(1, 0, 2)
                .reshape(P, NB_D * cn)
            )

        for j in range(NSPLIT):
            foff = j * FS
            w1e = np.ascontiguousarray(
                W1f[grp][:, :, foff : foff + FS]
                .astype(bf16)
                .reshape(NSLOT, NB_D, P, NB_F, P)
                .transpose(0, 2, 3, 1, 4)
            )
            w2e = np.ascontiguousarray(
                W2f[grp][:, foff : foff + FS, :]
                .astype(bf16)
                .reshape(NSLOT, NB_F, P, NB_D, P)
                .transpose(3, 2, 0, 1, 4)
            )
            b1e = np.ascontiguousarray(
                b1f[grp][:, foff : foff + FS]
                .reshape(NSLOT, NB_F, P)
                .transpose(2, 0, 1)
                .reshape(P, NSLOT * NB_F)
            )
            in_maps[g * NSPLIT + j] = {
                "x": x_arr,
                "w1": w1e,
                "w2": w2e,
                "b1": b1e,
            }

    global _last_in_maps
    _last_in_maps = in_maps

    res = run_bass_kernel_spmd(nc, in_maps, core_ids=list(range(N_CORES)))

    out = np.zeros((T, D), np.float32)
    b2f = np.asarray(b2, np.float32)
    for g, grp in enumerate(groups):
        acc = np.zeros((P, NB_D, C), np.float32)
        for j in range(NSPLIT):
            acc += np.asarray(res.results[g * NSPLIT + j]["out"], np.float32)
        for s, e in enumerate(grp):
            n = counts[e]
            y = acc[:, :, offs[s] : offs[s] + n].transpose(2, 1, 0).reshape(n, D)
            out[idxs[e]] += wgts[e][:, None] * y
            if b2f[e].any():
                out[idxs[e]] += wgts[e][:, None] * b2f[e][None, :]
    return out.reshape(B, S_, D_).astype(np.float32)
